# revision 29
# baseline (speedup 1.0000x reference)
"""PVNet-style RANSAC keypoint voting (EvalWrapper) on 8 Trainium2 cores.

Split of work:
  host (jax CPU, bitwise-identical to the reference):
      mask/argmax, categorical sampling, hypothesis generation, winner
      refinement, and exact inlier-count recompute for a small candidate set.
  device (Bass, 8 NeuronCores):
      the O(HYP*VN*N) voting loop. Core c handles image c//4, hypotheses
      (c%4)*128..+128. A pixel votes for hypothesis h iff
      cos(angle(h - pix, dir)) > 0.99, which is equivalent to
          num > 0 and q > 0, where
          num = (h - pix) . dn                      (linear in (hx, hy, 1))
          q   = num^2 - T^2 * (|h - pix|^2 + eps)   (quadratic in hyp coords)
      Both are contractions of per-pixel coefficient vectors against per-hyp
      monomial vectors -> TensorE matmuls (contraction dims 6 and 3). Each
      f32 factor is split into two bf16s (hi + lo) and the contraction dim is
      expanded 3x (hi*hi + hi*lo + lo*hi), giving ~1e-5 relative accuracy at
      full bf16 PE speed. VectorE takes min(q, num); ScalarE Sign+accumulate
      produces per-hyp sums of sign(min) = 2*count - nfg_pad.
      Only foreground pixels are shipped (background never votes).

  The device counts are exact up to a few ULPs worth of borderline-cos pixel
  flips (measured max |delta| = 3). The host takes every hypothesis within
  MARGIN of the device max and recomputes its exact count with the reference's
  own jnp expressions (verified bitwise-identical to the full computation), so
  the final argmax winner -- including lowest-index tie-breaks -- matches the
  reference exactly, and the refined output is bit-for-bit the reference's.
"""

import numpy as np
import ml_dtypes

B, H, W, VN = 2, 64, 64, 9
HYP = 512
T = 0.99
T2 = T * T
EPS = 1e-6
N = H * W
NCORES = 8
CORES_PER_IMG = 4
HYP_PER_CORE = HYP // CORES_PER_IMG  # 128
PG = 1024                # pixels per partition-block group
MARGIN = 8.0

_BF16 = ml_dtypes.bfloat16

_nc_cache = {}
last_exec_time_ns = None   # set when BASS_KERNEL_TRACE=1
last_results = None


# ----------------------------------------------------------------------------
# host-side jax pieces (bitwise-identical to reference.py)
# ----------------------------------------------------------------------------

def _host_funcs():
    import jax
    import jax.numpy as jnp

    cpu = jax.devices("cpu")[0]

    def _perp(d):
        return jnp.stack([d[..., 1], -d[..., 0]], axis=-1)

    def run_part1(seg_pred, vertex_pred):
        with jax.default_device(cpu):
            seg_pred = jnp.asarray(np.asarray(seg_pred))
            vertex_pred = jnp.asarray(np.asarray(vertex_pred))
            vp = jnp.transpose(vertex_pred, (0, 2, 3, 1)).reshape(B, H, W, VN, 2)
            mask = jnp.argmax(seg_pred, axis=1)
            ys, xs = jnp.meshgrid(jnp.arange(H), jnp.arange(W), indexing="ij")
            coords = jnp.stack([xs, ys], -1).reshape(-1, 2).astype(jnp.float32)
            mflat = mask.reshape(B, -1).astype(jnp.float32)
            dflat = vp.reshape(B, H * W, VN, 2)
            keys = jax.random.split(jax.random.key(42), B)

            def part1(m, direct, key):
                logits = jnp.where(m > 0, 0.0, -1e9)
                idxs = jax.random.categorical(key, logits, shape=(HYP, VN, 2))
                ar = jnp.arange(VN)[None, :]
                p0 = coords[idxs[..., 0]]
                p1 = coords[idxs[..., 1]]
                d0 = direct[idxs[..., 0], ar]
                d1 = direct[idxs[..., 1], ar]
                n0, n1 = _perp(d0), _perp(d1)
                det = n0[..., 0] * n1[..., 1] - n0[..., 1] * n1[..., 0]
                det_s = jnp.where(jnp.abs(det) < EPS, EPS, det)
                c0 = jnp.sum(n0 * p0, -1)
                c1 = jnp.sum(n1 * p1, -1)
                hx = (c0 * n1[..., 1] - c1 * n0[..., 1]) / det_s
                hy = (n0[..., 0] * c1 - n1[..., 0] * c0) / det_s
                hyp = jnp.stack([hx, hy], -1)
                dn = direct / jnp.sqrt(jnp.sum(direct * direct, -1, keepdims=True) + EPS)
                return hyp, dn

            hyp_all, dn_all = jax.vmap(part1)(mflat, dflat, keys)
            return (np.asarray(hyp_all), np.asarray(dn_all), np.asarray(mflat),
                    np.asarray(dflat), np.asarray(coords))

    def exact_counts(hyp_c, dn_v, m, coords):
        """Reference-bitwise inlier counts for a subset of hyps of one (b, vn).

        hyp_c: [K, 2] f32, dn_v: [N, 2] f32, m: [N] f32 -> [K] f32
        """
        with jax.default_device(cpu):
            hyp_c = jnp.asarray(hyp_c)
            dn_v = jnp.asarray(dn_v)
            m = jnp.asarray(m)
            coords = jnp.asarray(coords)
            diff = hyp_c[:, None, :] - coords[None, :, :]
            dist = jnp.sqrt(jnp.sum(diff * diff, -1) + EPS)
            cos = jnp.einsum("knc,nc->kn", diff, dn_v) / dist
            inlier = (cos > T).astype(jnp.float32)
            return np.asarray(jnp.einsum("kn,n->k", inlier, m))

    def run_part3(hyp_all, dflat, dn_all, mflat, coords, win_idx):
        with jax.default_device(cpu):
            hyp_all = jnp.asarray(hyp_all)
            dflat = jnp.asarray(dflat)
            dn_all = jnp.asarray(dn_all)
            mflat = jnp.asarray(mflat)
            coords = jnp.asarray(coords)
            win_idx = jnp.asarray(win_idx)

            def part3(hyp, direct, dn, m, wi):
                win = hyp[wi, jnp.arange(VN)]
                dw = win[None, :, :] - coords[:, None, :]
                dwn = jnp.sqrt(jnp.sum(dw * dw, -1) + EPS)
                cw = jnp.sum(dw * dn, -1) / dwn
                wgt = ((cw > T) & (m[:, None] > 0)).astype(jnp.float32)
                normal = _perp(direct) * wgt[..., None]
                bvec = jnp.sum(normal * coords[:, None, :], -1)
                ATA = jnp.einsum("nvc,nvd->vcd", normal, normal)
                ATb = jnp.einsum("nvc,nv->vc", normal, bvec)
                detA = ATA[:, 0, 0] * ATA[:, 1, 1] - ATA[:, 0, 1] * ATA[:, 1, 0]
                detA_s = jnp.where(jnp.abs(detA) < EPS, EPS, detA)
                px = (ATA[:, 1, 1] * ATb[:, 0] - ATA[:, 0, 1] * ATb[:, 1]) / detA_s
                py = (-ATA[:, 1, 0] * ATb[:, 0] + ATA[:, 0, 0] * ATb[:, 1]) / detA_s
                refined = jnp.stack([px, py], -1)
                ok = (jnp.abs(detA) >= EPS)[:, None]
                return jnp.where(ok, refined, win)

            return np.asarray(jax.vmap(part3)(hyp_all, dflat, dn_all, mflat, win_idx))

    return run_part1, exact_counts, run_part3


# ----------------------------------------------------------------------------
# bf16x2 split packing
# ----------------------------------------------------------------------------

def _split_bf16(x):
    """f32/f64 array -> (hi, lo) bf16 arrays with hi + lo ~= f32(x)."""
    x32 = np.asarray(x, np.float32)
    hi = x32.astype(_BF16)
    lo = (x32 - hi.astype(np.float32)).astype(_BF16)
    return hi, lo


def _expand_rows(coeff, mono):
    """coeff [R, X], mono [R, Y] -> (rhs [3R, X], lhs [3R, Y]) bf16 such that
    sum_r lhs[r].T * rhs[r] ~= sum coeff*mono with ~2^-17 relative error:
    hi*hi + lo*hi + hi*lo (the lo*lo term is dropped)."""
    R, X = coeff.shape
    Y = mono.shape[1]
    ch, cl = _split_bf16(coeff)
    mh, ml = _split_bf16(mono)
    rhs = np.empty((3 * R, X), _BF16)
    lhs = np.empty((3 * R, Y), _BF16)
    rhs[0::3], rhs[1::3], rhs[2::3] = ch, cl, ch
    lhs[0::3], lhs[1::3], lhs[2::3] = mh, mh, ml
    return rhs, lhs


# ----------------------------------------------------------------------------
# device program
# ----------------------------------------------------------------------------

QROWS = 18   # 6 logical q-coefficients x3 expansion
NROWS = 9    # 3 logical num-coefficients x3 expansion
CROWS = QROWS + NROWS  # 27-row contraction block per pixel group

# Partition layout (AP base partitions must be in {0, 32, 64}, and the DVE
# cannot read two PSUM operands in one op):
#   rows  0..17  q-coefficient rows, pixel group 0      (lhsT: q-monomials)
#   rows 32..49  q-coefficient rows, pixel group 1      (lhsT: q-monomials)
#   rows 64..72  num-coefficient rows, both groups      (lhsT: num-monomials)
# q matmuls contract 18 rows, num matmuls 9 rows; num groups are told apart
# by column ranges. Counting: ScalarE emits rn = relu(num) (PSUM->SBUF bf16),
# then one VectorE scalar_tensor_tensor computes
#   out = (q is_gt 0) logical_and rn  in {0.0, 1.0},  accum_out = count.
# rhs/lhs DMAs are batched 3 vns per transfer (the ~2us fixed cost per
# dma_start dominates small transfers).

MMC = 512                     # columns per matmul instruction (one PSUM bank)

def _build_bass(nfg_pad):
    import concourse.mybir as mybir
    from concourse import bacc, tile

    G = nfg_pad // PG            # pixel groups (pairs: g even at rows 0, odd at 32)
    assert nfg_pad % (2 * PG) == 0, nfg_pad
    NMM = PG // MMC              # matmuls per (group, q|num)

    nc = bacc.Bacc("TRN2", target_bir_lowering=False, debug=False)
    lhs_d = nc.declare_dram_parameter("lhs", [128, VN * 128], mybir.dt.bfloat16,
                                      isOutput=False)
    rq0_d = nc.declare_dram_parameter("rhsq0", [QROWS, VN * (G // 2) * PG],
                                      mybir.dt.bfloat16, isOutput=False)
    rq1_d = nc.declare_dram_parameter("rhsq1", [QROWS, VN * (G // 2) * PG],
                                      mybir.dt.bfloat16, isOutput=False)
    rn_d = nc.declare_dram_parameter("rhsn", [NROWS, VN * G * PG],
                                     mybir.dt.bfloat16, isOutput=False)
    acc_d = nc.declare_dram_parameter("acc", [128, VN * G], mybir.dt.float32,
                                      isOutput=True)

    QCOLS = (G // 2) * PG        # q columns per vn per parity block
    NCOLS = G * PG               # num columns per vn
    VB = 3                       # vns per DMA batch
    assert VN % VB == 0

    with tile.TileContext(nc) as tc:
        with (
            tc.tile_pool(name="const", bufs=1) as cpool,
            tc.tile_pool(name="rhs", bufs=2) as rpool,
            tc.tile_pool(name="sn", bufs=4) as snpool,
            tc.tile_pool(name="pr", bufs=4) as prpool,
            tc.tile_pool(name="psq", bufs=2, space="PSUM") as psq,
            tc.tile_pool(name="psn", bufs=2, space="PSUM") as psn,
        ):
            lhs_t = cpool.tile([128, VN * 128], mybir.dt.bfloat16)
            nc.sync.dma_start(lhs_t[0:73, :], lhs_d[0:73, :])
            acc_t = cpool.tile([128, VN * G], mybir.dt.float32)

            for vb in range(VN // VB):
                rhs_t = rpool.tile([128, VB * (QCOLS + NCOLS)], mybir.dt.bfloat16)
                qs = slice(vb * VB * QCOLS, (vb + 1) * VB * QCOLS)
                ns = slice(vb * VB * NCOLS, (vb + 1) * VB * NCOLS)
                nc.sync.dma_start(rhs_t[0:QROWS, 0:VB * QCOLS], rq0_d[:, qs])
                nc.sync.dma_start(rhs_t[32:32 + QROWS, 0:VB * QCOLS], rq1_d[:, qs])
                nc.sync.dma_start(
                    rhs_t[64:64 + NROWS, VB * QCOLS:VB * (QCOLS + NCOLS)],
                    rn_d[:, ns])
                for j in range(VB):
                    vn = vb * VB + j
                    for g in range(G):
                        qb = 32 * (g % 2)
                        qco = j * QCOLS + (g // 2) * PG
                        nco = VB * QCOLS + j * NCOLS + g * PG
                        qp = psq.tile([128, PG], mybir.dt.float32)
                        np_ = psn.tile([128, PG], mybir.dt.float32)
                        for s in range(NMM):
                            nc.tensor.matmul(
                                qp[:, s * MMC:(s + 1) * MMC],
                                lhs_t[qb:qb + QROWS, vn * 128:(vn + 1) * 128],
                                rhs_t[qb:qb + QROWS,
                                      qco + s * MMC:qco + (s + 1) * MMC],
                                start=True, stop=True)
                            nc.tensor.matmul(
                                np_[:, s * MMC:(s + 1) * MMC],
                                lhs_t[64:64 + NROWS, vn * 128:(vn + 1) * 128],
                                rhs_t[64:64 + NROWS,
                                      nco + s * MMC:nco + (s + 1) * MMC],
                                start=True, stop=True)
                        slot = vn * G + g
                        sn_t = snpool.tile([128, PG], mybir.dt.bfloat16)
                        nc.scalar.activation(sn_t[:], np_[:],
                                             mybir.ActivationFunctionType.Relu)
                        st_t = prpool.tile([128, PG], mybir.dt.bfloat16)
                        nc.vector.scalar_tensor_tensor(
                            st_t[:], qp[:], 0.0, sn_t[:],
                            mybir.AluOpType.is_gt, mybir.AluOpType.logical_and,
                            accum_out=acc_t[:, slot:slot + 1])
            nc.sync.dma_start(acc_d[:], acc_t[:])
    return nc


def _get_nc(nfg_pad):
    if nfg_pad not in _nc_cache:
        nc = _build_bass(nfg_pad)
        if not nc.is_finalized():
            nc.finalize()
        _nc_cache[nfg_pad] = nc
    return _nc_cache[nfg_pad]


# ----------------------------------------------------------------------------
# per-core input packing
# ----------------------------------------------------------------------------

def _pack_image_rhs(fg_idx, coords, dn_img, nfg_pad):
    """rhs for one image -> dict with rhsq0 [18, VN*QCOLS], rhsq1 [18, VN*QCOLS],
    rhsn [9, VN*NCOLS] bf16."""
    G = nfg_pad // PG
    QCOLS = (G // 2) * PG
    NCOLS = G * PG
    nfg = len(fg_idx)
    cx = coords[fg_idx, 0].astype(np.float64)
    cy = coords[fg_idx, 1].astype(np.float64)

    rq0 = np.zeros((QROWS, VN * QCOLS), _BF16)
    rq1 = np.zeros((QROWS, VN * QCOLS), _BF16)
    rn = np.zeros((NROWS, VN * NCOLS), _BF16)
    for vn in range(VN):
        A = dn_img[fg_idx, vn, 0].astype(np.float64)
        Bc = dn_img[fg_idx, vn, 1].astype(np.float64)
        C = -(cx * A + cy * Bc)
        q6 = np.zeros((6, nfg_pad), np.float64)
        n3 = np.zeros((3, nfg_pad), np.float64)
        q6[0, :nfg] = A * A - T2
        q6[1, :nfg] = 2.0 * A * Bc
        q6[2, :nfg] = Bc * Bc - T2
        q6[3, :nfg] = 2.0 * A * C + 2.0 * T2 * cx
        q6[4, :nfg] = 2.0 * Bc * C + 2.0 * T2 * cy
        q6[5, :nfg] = C * C - T2 * (cx * cx + cy * cy + EPS)
        n3[0, :nfg] = A
        n3[1, :nfg] = Bc
        n3[2, :nfg] = C
        # padding pixels: q = -1, num = -1  (constant rows; monomial row = 1)
        q6[5, nfg:] = -1.0
        n3[2, nfg:] = -1.0

        rhs_q, _ = _expand_rows(q6, np.zeros((6, 1)))   # [18, nfg_pad]
        rhs_n, _ = _expand_rows(n3, np.zeros((3, 1)))   # [9, nfg_pad]
        for g in range(G):
            sl = slice(g * PG, (g + 1) * PG)
            dst = rq0 if g % 2 == 0 else rq1
            qc = vn * QCOLS + (g // 2) * PG
            dst[:, qc:qc + PG] = rhs_q[:, sl]
            nc_ = vn * NCOLS + g * PG
            rn[:, nc_:nc_ + PG] = rhs_n[:, sl]
    return {"rhsq0": rq0, "rhsq1": rq1, "rhsn": rn}


def _pack_core_lhs(hyp_blk):
    """lhs for one core: [128, VN * 128] bf16. hyp_blk: [128, VN, 2] f32."""
    lhs_all = np.zeros((128, VN * 128), _BF16)
    for vn in range(VN):
        hx = hyp_blk[:, vn, 0].astype(np.float64)
        hy = hyp_blk[:, vn, 1].astype(np.float64)
        one = np.ones_like(hx)
        m6 = np.stack([hx * hx, hx * hy, hy * hy, hx, hy, one])   # [6, 128]
        m3 = np.stack([hx, hy, one])                              # [3, 128]
        _, lhs_q = _expand_rows(np.zeros((6, 1)), m6)
        _, lhs_n = _expand_rows(np.zeros((3, 1)), m3)
        lhs_all[0:QROWS, vn * 128:(vn + 1) * 128] = lhs_q
        lhs_all[32:32 + QROWS, vn * 128:(vn + 1) * 128] = lhs_q
        lhs_all[64:64 + NROWS, vn * 128:(vn + 1) * 128] = lhs_n
    return lhs_all


# ----------------------------------------------------------------------------
# main entry
# ----------------------------------------------------------------------------

def kernel(seg_pred, vertex_pred):
    global last_exec_time_ns, last_results
    import os

    run_part1, exact_counts, run_part3 = _host_funcs()
    hyp_all, dn_all, mflat, dflat, coords = run_part1(seg_pred, vertex_pred)

    fg = [np.nonzero(mflat[b] > 0)[0] for b in range(B)]
    maxfg = max(len(f) for f in fg)
    nfg_pad = max(2048, ((maxfg + 2047) // 2048) * 2048)
    G = nfg_pad // PG
    NACC = G

    nc = _get_nc(nfg_pad)

    rhs_img = [_pack_image_rhs(fg[b], coords, dn_all[b], nfg_pad) for b in range(B)]
    in_maps = []
    for c in range(NCORES):
        b = c // CORES_PER_IMG
        k = c % CORES_PER_IMG
        hyp_blk = hyp_all[b, k * HYP_PER_CORE:(k + 1) * HYP_PER_CORE]
        in_maps.append({"lhs": np.ascontiguousarray(_pack_core_lhs(hyp_blk)),
                        **rhs_img[b]})

    from concourse.bass_utils import run_bass_kernel_spmd
    trace = os.environ.get("BASS_KERNEL_TRACE", "0") == "1"
    res = run_bass_kernel_spmd(nc, in_maps, core_ids=list(range(NCORES)),
                               trace=trace)
    last_exec_time_ns = res.exec_time_ns
    last_results = res

    # assemble device counts [B, HYP, VN]: acc slots are direct counts per group
    dev_counts = np.zeros((B, HYP, VN), np.float64)
    for c in range(NCORES):
        b = c // CORES_PER_IMG
        k = c % CORES_PER_IMG
        acc = np.asarray(res.results[c]["acc"], np.float64)  # [128, VN*G]
        dev_counts[b, k * HYP_PER_CORE:(k + 1) * HYP_PER_CORE] = \
            acc.reshape(128, VN, G).sum(-1)

    # winner selection: exact recompute of candidates (reference-bitwise)
    win_idx = np.zeros((B, VN), np.int64)
    for b in range(B):
        for vn in range(VN):
            dc = dev_counts[b, :, vn]
            cand = np.nonzero(dc >= dc.max() - MARGIN)[0]
            cnt = exact_counts(hyp_all[b][cand, vn, :], dn_all[b][:, vn, :],
                               mflat[b], coords)
            win_idx[b, vn] = cand[int(np.argmax(cnt))]

    out = run_part3(hyp_all, dflat, dn_all, mflat, coords, win_idx)
    return np.asarray(out, np.float32)


# revision 33
# speedup vs baseline: 1.0195x; 1.0195x over previous
"""PVNet-style RANSAC keypoint voting (EvalWrapper) on 8 Trainium2 cores.

Split of work:
  host (jax CPU, bitwise-identical to the reference):
      mask/argmax, categorical sampling, hypothesis generation, winner
      refinement, and exact inlier-count recompute for a small candidate set.
  device (Bass, 8 NeuronCores):
      the O(HYP*VN*N) voting loop. Core c handles image c//4, hypotheses
      (c%4)*128..+128. A pixel votes for hypothesis h iff
      cos(angle(h - pix, dir)) > 0.99, which is equivalent to
          num > 0 and q > 0, where
          num = (h - pix) . dn                      (linear in (hx, hy, 1))
          q   = num^2 - T^2 * (|h - pix|^2 + eps)   (quadratic in hyp coords)
      Both are contractions of per-pixel coefficient vectors against per-hyp
      monomial vectors -> TensorE matmuls (contraction dims 6 and 3). Each
      f32 factor is split into two bf16s (hi + lo) and the contraction dim is
      expanded 3x (hi*hi + hi*lo + lo*hi), giving ~1e-5 relative accuracy at
      full bf16 PE speed. VectorE takes min(q, num); ScalarE Sign+accumulate
      produces per-hyp sums of sign(min) = 2*count - nfg_pad.
      Only foreground pixels are shipped (background never votes).

  The device counts are exact up to a few ULPs worth of borderline-cos pixel
  flips (measured max |delta| = 3). The host takes every hypothesis within
  MARGIN of the device max and recomputes its exact count with the reference's
  own jnp expressions (verified bitwise-identical to the full computation), so
  the final argmax winner -- including lowest-index tie-breaks -- matches the
  reference exactly, and the refined output is bit-for-bit the reference's.
"""

import numpy as np
import ml_dtypes

B, H, W, VN = 2, 64, 64, 9
HYP = 512
T = 0.99
T2 = T * T
EPS = 1e-6
N = H * W
NCORES = 8
CORES_PER_IMG = 4
HYP_PER_CORE = HYP // CORES_PER_IMG  # 128
PG = 1024                # pixels per partition-block group
MARGIN = 8.0

_BF16 = ml_dtypes.bfloat16

_nc_cache = {}
last_exec_time_ns = None   # set when BASS_KERNEL_TRACE=1
last_results = None


# ----------------------------------------------------------------------------
# host-side jax pieces (bitwise-identical to reference.py)
# ----------------------------------------------------------------------------

def _host_funcs():
    import jax
    import jax.numpy as jnp

    cpu = jax.devices("cpu")[0]

    def _perp(d):
        return jnp.stack([d[..., 1], -d[..., 0]], axis=-1)

    def run_part1(seg_pred, vertex_pred):
        with jax.default_device(cpu):
            seg_pred = jnp.asarray(np.asarray(seg_pred))
            vertex_pred = jnp.asarray(np.asarray(vertex_pred))
            vp = jnp.transpose(vertex_pred, (0, 2, 3, 1)).reshape(B, H, W, VN, 2)
            mask = jnp.argmax(seg_pred, axis=1)
            ys, xs = jnp.meshgrid(jnp.arange(H), jnp.arange(W), indexing="ij")
            coords = jnp.stack([xs, ys], -1).reshape(-1, 2).astype(jnp.float32)
            mflat = mask.reshape(B, -1).astype(jnp.float32)
            dflat = vp.reshape(B, H * W, VN, 2)
            keys = jax.random.split(jax.random.key(42), B)

            def part1(m, direct, key):
                logits = jnp.where(m > 0, 0.0, -1e9)
                idxs = jax.random.categorical(key, logits, shape=(HYP, VN, 2))
                ar = jnp.arange(VN)[None, :]
                p0 = coords[idxs[..., 0]]
                p1 = coords[idxs[..., 1]]
                d0 = direct[idxs[..., 0], ar]
                d1 = direct[idxs[..., 1], ar]
                n0, n1 = _perp(d0), _perp(d1)
                det = n0[..., 0] * n1[..., 1] - n0[..., 1] * n1[..., 0]
                det_s = jnp.where(jnp.abs(det) < EPS, EPS, det)
                c0 = jnp.sum(n0 * p0, -1)
                c1 = jnp.sum(n1 * p1, -1)
                hx = (c0 * n1[..., 1] - c1 * n0[..., 1]) / det_s
                hy = (n0[..., 0] * c1 - n1[..., 0] * c0) / det_s
                hyp = jnp.stack([hx, hy], -1)
                dn = direct / jnp.sqrt(jnp.sum(direct * direct, -1, keepdims=True) + EPS)
                return hyp, dn

            hyp_all, dn_all = jax.vmap(part1)(mflat, dflat, keys)
            return (np.asarray(hyp_all), np.asarray(dn_all), np.asarray(mflat),
                    np.asarray(dflat), np.asarray(coords))

    def exact_counts(hyp_c, dn_v, m, coords):
        """Reference-bitwise inlier counts for a subset of hyps of one (b, vn).

        hyp_c: [K, 2] f32, dn_v: [N, 2] f32, m: [N] f32 -> [K] f32
        """
        with jax.default_device(cpu):
            hyp_c = jnp.asarray(hyp_c)
            dn_v = jnp.asarray(dn_v)
            m = jnp.asarray(m)
            coords = jnp.asarray(coords)
            diff = hyp_c[:, None, :] - coords[None, :, :]
            dist = jnp.sqrt(jnp.sum(diff * diff, -1) + EPS)
            cos = jnp.einsum("knc,nc->kn", diff, dn_v) / dist
            inlier = (cos > T).astype(jnp.float32)
            return np.asarray(jnp.einsum("kn,n->k", inlier, m))

    def run_part3(hyp_all, dflat, dn_all, mflat, coords, win_idx):
        with jax.default_device(cpu):
            hyp_all = jnp.asarray(hyp_all)
            dflat = jnp.asarray(dflat)
            dn_all = jnp.asarray(dn_all)
            mflat = jnp.asarray(mflat)
            coords = jnp.asarray(coords)
            win_idx = jnp.asarray(win_idx)

            def part3(hyp, direct, dn, m, wi):
                win = hyp[wi, jnp.arange(VN)]
                dw = win[None, :, :] - coords[:, None, :]
                dwn = jnp.sqrt(jnp.sum(dw * dw, -1) + EPS)
                cw = jnp.sum(dw * dn, -1) / dwn
                wgt = ((cw > T) & (m[:, None] > 0)).astype(jnp.float32)
                normal = _perp(direct) * wgt[..., None]
                bvec = jnp.sum(normal * coords[:, None, :], -1)
                ATA = jnp.einsum("nvc,nvd->vcd", normal, normal)
                ATb = jnp.einsum("nvc,nv->vc", normal, bvec)
                detA = ATA[:, 0, 0] * ATA[:, 1, 1] - ATA[:, 0, 1] * ATA[:, 1, 0]
                detA_s = jnp.where(jnp.abs(detA) < EPS, EPS, detA)
                px = (ATA[:, 1, 1] * ATb[:, 0] - ATA[:, 0, 1] * ATb[:, 1]) / detA_s
                py = (-ATA[:, 1, 0] * ATb[:, 0] + ATA[:, 0, 0] * ATb[:, 1]) / detA_s
                refined = jnp.stack([px, py], -1)
                ok = (jnp.abs(detA) >= EPS)[:, None]
                return jnp.where(ok, refined, win)

            return np.asarray(jax.vmap(part3)(hyp_all, dflat, dn_all, mflat, win_idx))

    return run_part1, exact_counts, run_part3


# ----------------------------------------------------------------------------
# bf16x2 split packing
# ----------------------------------------------------------------------------

def _split_bf16(x):
    """f32/f64 array -> (hi, lo) bf16 arrays with hi + lo ~= f32(x)."""
    x32 = np.asarray(x, np.float32)
    hi = x32.astype(_BF16)
    lo = (x32 - hi.astype(np.float32)).astype(_BF16)
    return hi, lo


def _expand_rows(coeff, mono):
    """coeff [R, X], mono [R, Y] -> (rhs [3R, X], lhs [3R, Y]) bf16 such that
    sum_r lhs[r].T * rhs[r] ~= sum coeff*mono with ~2^-17 relative error:
    hi*hi + lo*hi + hi*lo (the lo*lo term is dropped)."""
    R, X = coeff.shape
    Y = mono.shape[1]
    ch, cl = _split_bf16(coeff)
    mh, ml = _split_bf16(mono)
    rhs = np.empty((3 * R, X), _BF16)
    lhs = np.empty((3 * R, Y), _BF16)
    rhs[0::3], rhs[1::3], rhs[2::3] = ch, cl, ch
    lhs[0::3], lhs[1::3], lhs[2::3] = mh, mh, ml
    return rhs, lhs


# ----------------------------------------------------------------------------
# device program
# ----------------------------------------------------------------------------

QROWS = 18   # 6 logical q-coefficients x3 bf16x2 expansion
NROWS = 3    # 3 logical num-coefficients, single bf16 (only its sign is used,
             # and it only matters when |num| > 0.99*dist >> bf16 error)
CROWS = QROWS + NROWS

# Partition layout (AP base partitions must be in {0, 32, 64}, and the DVE
# cannot read two PSUM operands in one op):
#   rows  0..17  q-coefficient rows, pixel group 0      (lhsT: q-monomials)
#   rows 32..49  q-coefficient rows, pixel group 1      (lhsT: q-monomials)
#   rows 64..72  num-coefficient rows, both groups      (lhsT: num-monomials)
# q matmuls contract 18 rows, num matmuls 9 rows; num groups are told apart
# by column ranges. Counting: ScalarE emits rn = relu(num) (PSUM->SBUF bf16),
# then one VectorE scalar_tensor_tensor computes
#   out = (q is_gt 0) logical_and rn  in {0.0, 1.0},  accum_out = count.
# rhs/lhs DMAs are batched 3 vns per transfer (the ~2us fixed cost per
# dma_start dominates small transfers).

MMC = 512                     # columns per matmul instruction (one PSUM bank)

def _build_bass(nfg_pad):
    import concourse.mybir as mybir
    from concourse import bacc, tile

    G = nfg_pad // PG            # pixel groups (pairs: g even at rows 0, odd at 32)
    assert nfg_pad % (2 * PG) == 0, nfg_pad
    NMM = PG // MMC              # matmuls per (group, q|num)

    nc = bacc.Bacc("TRN2", target_bir_lowering=False, debug=False)
    lhs_d = nc.declare_dram_parameter("lhs", [128, VN * 128], mybir.dt.bfloat16,
                                      isOutput=False)
    rq0_d = nc.declare_dram_parameter("rhsq0", [QROWS, VN * (G // 2) * PG],
                                      mybir.dt.bfloat16, isOutput=False)
    rq1_d = nc.declare_dram_parameter("rhsq1", [QROWS, VN * (G // 2) * PG],
                                      mybir.dt.bfloat16, isOutput=False)
    rn_d = nc.declare_dram_parameter("rhsn", [NROWS, VN * G * PG],
                                     mybir.dt.bfloat16, isOutput=False)
    acc_d = nc.declare_dram_parameter("acc", [128, VN * G], mybir.dt.float32,
                                      isOutput=True)

    QCOLS = (G // 2) * PG        # q columns per vn per parity block
    NCOLS = G * PG               # num columns per vn
    VB = 3                       # vns per DMA batch
    assert VN % VB == 0

    with tile.TileContext(nc) as tc:
        with (
            tc.tile_pool(name="const", bufs=1) as cpool,
            tc.tile_pool(name="rhs", bufs=2) as rpool,
            tc.tile_pool(name="sn", bufs=4) as snpool,
            tc.tile_pool(name="pr", bufs=4) as prpool,
            tc.tile_pool(name="psq", bufs=2, space="PSUM") as psq,
            tc.tile_pool(name="psn", bufs=2, space="PSUM") as psn,
        ):
            lhs_t = cpool.tile([128, VN * 128], mybir.dt.bfloat16)
            nc.sync.dma_start(lhs_t[0:73, :], lhs_d[0:73, :])
            acc_t = cpool.tile([128, VN * G], mybir.dt.float32)

            for vb in range(VN // VB):
                rhs_t = rpool.tile([128, VB * (QCOLS + NCOLS)], mybir.dt.bfloat16)
                qs = slice(vb * VB * QCOLS, (vb + 1) * VB * QCOLS)
                ns = slice(vb * VB * NCOLS, (vb + 1) * VB * NCOLS)
                nc.sync.dma_start(rhs_t[0:QROWS, 0:VB * QCOLS], rq0_d[:, qs])
                nc.sync.dma_start(
                    rhs_t[64:64 + NROWS, VB * QCOLS:VB * (QCOLS + NCOLS)],
                    rn_d[:, ns])
                # second HWDGE ring (Act) in parallel with the Sync ring
                nc.scalar.dma_start(rhs_t[32:32 + QROWS, 0:VB * QCOLS],
                                    rq1_d[:, qs])
                for j in range(VB):
                    vn = vb * VB + j
                    for g in range(G):
                        qb = 32 * (g % 2)
                        qco = j * QCOLS + (g // 2) * PG
                        nco = VB * QCOLS + j * NCOLS + g * PG
                        qp = psq.tile([128, PG], mybir.dt.float32)
                        np_ = psn.tile([128, PG], mybir.dt.float32)
                        for s in range(NMM):
                            nc.tensor.matmul(
                                qp[:, s * MMC:(s + 1) * MMC],
                                lhs_t[qb:qb + QROWS, vn * 128:(vn + 1) * 128],
                                rhs_t[qb:qb + QROWS,
                                      qco + s * MMC:qco + (s + 1) * MMC],
                                start=True, stop=True)
                            nc.tensor.matmul(
                                np_[:, s * MMC:(s + 1) * MMC],
                                lhs_t[64:64 + NROWS, vn * 128:(vn + 1) * 128],
                                rhs_t[64:64 + NROWS,
                                      nco + s * MMC:nco + (s + 1) * MMC],
                                start=True, stop=True)
                        slot = vn * G + g
                        sn_t = snpool.tile([128, PG], mybir.dt.bfloat16)
                        nc.scalar.activation(sn_t[:], np_[:],
                                             mybir.ActivationFunctionType.Relu)
                        st_t = prpool.tile([128, PG], mybir.dt.bfloat16)
                        nc.vector.scalar_tensor_tensor(
                            st_t[:], qp[:], 0.0, sn_t[:],
                            mybir.AluOpType.is_gt, mybir.AluOpType.logical_and,
                            accum_out=acc_t[:, slot:slot + 1])
            nc.sync.dma_start(acc_d[:], acc_t[:])
    return nc


def _get_nc(nfg_pad):
    if nfg_pad not in _nc_cache:
        nc = _build_bass(nfg_pad)
        if not nc.is_finalized():
            nc.finalize()
        _nc_cache[nfg_pad] = nc
    return _nc_cache[nfg_pad]


# ----------------------------------------------------------------------------
# per-core input packing
# ----------------------------------------------------------------------------

def _pack_image_rhs(fg_idx, coords, dn_img, nfg_pad):
    """rhs for one image -> dict with rhsq0 [18, VN*QCOLS], rhsq1 [18, VN*QCOLS],
    rhsn [9, VN*NCOLS] bf16."""
    G = nfg_pad // PG
    QCOLS = (G // 2) * PG
    NCOLS = G * PG
    nfg = len(fg_idx)
    cx = coords[fg_idx, 0].astype(np.float64)
    cy = coords[fg_idx, 1].astype(np.float64)

    rq0 = np.zeros((QROWS, VN * QCOLS), _BF16)
    rq1 = np.zeros((QROWS, VN * QCOLS), _BF16)
    rn = np.zeros((NROWS, VN * NCOLS), _BF16)
    for vn in range(VN):
        A = dn_img[fg_idx, vn, 0].astype(np.float64)
        Bc = dn_img[fg_idx, vn, 1].astype(np.float64)
        C = -(cx * A + cy * Bc)
        q6 = np.zeros((6, nfg_pad), np.float64)
        n3 = np.zeros((3, nfg_pad), np.float64)
        q6[0, :nfg] = A * A - T2
        q6[1, :nfg] = 2.0 * A * Bc
        q6[2, :nfg] = Bc * Bc - T2
        q6[3, :nfg] = 2.0 * A * C + 2.0 * T2 * cx
        q6[4, :nfg] = 2.0 * Bc * C + 2.0 * T2 * cy
        q6[5, :nfg] = C * C - T2 * (cx * cx + cy * cy + EPS)
        n3[0, :nfg] = A
        n3[1, :nfg] = Bc
        n3[2, :nfg] = C
        # padding pixels: q = -1, num = -1  (constant rows; monomial row = 1)
        q6[5, nfg:] = -1.0
        n3[2, nfg:] = -1.0

        rhs_q, _ = _expand_rows(q6, np.zeros((6, 1)))   # [18, nfg_pad]
        rhs_n = n3.astype(np.float32).astype(_BF16)     # [3, nfg_pad]
        for g in range(G):
            sl = slice(g * PG, (g + 1) * PG)
            dst = rq0 if g % 2 == 0 else rq1
            qc = vn * QCOLS + (g // 2) * PG
            dst[:, qc:qc + PG] = rhs_q[:, sl]
            nc_ = vn * NCOLS + g * PG
            rn[:, nc_:nc_ + PG] = rhs_n[:, sl]
    return {"rhsq0": rq0, "rhsq1": rq1, "rhsn": rn}


def _pack_core_lhs(hyp_blk):
    """lhs for one core: [128, VN * 128] bf16. hyp_blk: [128, VN, 2] f32."""
    lhs_all = np.zeros((128, VN * 128), _BF16)
    for vn in range(VN):
        hx = hyp_blk[:, vn, 0].astype(np.float64)
        hy = hyp_blk[:, vn, 1].astype(np.float64)
        one = np.ones_like(hx)
        m6 = np.stack([hx * hx, hx * hy, hy * hy, hx, hy, one])   # [6, 128]
        m3 = np.stack([hx, hy, one])                              # [3, 128]
        _, lhs_q = _expand_rows(np.zeros((6, 1)), m6)
        lhs_n = m3.astype(np.float32).astype(_BF16)
        lhs_all[0:QROWS, vn * 128:(vn + 1) * 128] = lhs_q
        lhs_all[32:32 + QROWS, vn * 128:(vn + 1) * 128] = lhs_q
        lhs_all[64:64 + NROWS, vn * 128:(vn + 1) * 128] = lhs_n
    return lhs_all


# ----------------------------------------------------------------------------
# main entry
# ----------------------------------------------------------------------------

def kernel(seg_pred, vertex_pred):
    global last_exec_time_ns, last_results
    import os

    run_part1, exact_counts, run_part3 = _host_funcs()
    hyp_all, dn_all, mflat, dflat, coords = run_part1(seg_pred, vertex_pred)

    fg = [np.nonzero(mflat[b] > 0)[0] for b in range(B)]
    maxfg = max(len(f) for f in fg)
    nfg_pad = max(2048, ((maxfg + 2047) // 2048) * 2048)
    G = nfg_pad // PG
    NACC = G

    nc = _get_nc(nfg_pad)

    rhs_img = [_pack_image_rhs(fg[b], coords, dn_all[b], nfg_pad) for b in range(B)]
    in_maps = []
    for c in range(NCORES):
        b = c // CORES_PER_IMG
        k = c % CORES_PER_IMG
        hyp_blk = hyp_all[b, k * HYP_PER_CORE:(k + 1) * HYP_PER_CORE]
        in_maps.append({"lhs": np.ascontiguousarray(_pack_core_lhs(hyp_blk)),
                        **rhs_img[b]})

    from concourse.bass_utils import run_bass_kernel_spmd
    trace = os.environ.get("BASS_KERNEL_TRACE", "0") == "1"
    res = run_bass_kernel_spmd(nc, in_maps, core_ids=list(range(NCORES)),
                               trace=trace)
    last_exec_time_ns = res.exec_time_ns
    last_results = res

    # assemble device counts [B, HYP, VN]: acc slots are direct counts per group
    dev_counts = np.zeros((B, HYP, VN), np.float64)
    for c in range(NCORES):
        b = c // CORES_PER_IMG
        k = c % CORES_PER_IMG
        acc = np.asarray(res.results[c]["acc"], np.float64)  # [128, VN*G]
        dev_counts[b, k * HYP_PER_CORE:(k + 1) * HYP_PER_CORE] = \
            acc.reshape(128, VN, G).sum(-1)

    # winner selection: exact recompute of candidates (reference-bitwise)
    win_idx = np.zeros((B, VN), np.int64)
    for b in range(B):
        for vn in range(VN):
            dc = dev_counts[b, :, vn]
            cand = np.nonzero(dc >= dc.max() - MARGIN)[0]
            cnt = exact_counts(hyp_all[b][cand, vn, :], dn_all[b][:, vn, :],
                               mflat[b], coords)
            win_idx[b, vn] = cand[int(np.argmax(cnt))]

    out = run_part3(hyp_all, dflat, dn_all, mflat, coords, win_idx)
    return np.asarray(out, np.float32)


# revision 34
# speedup vs baseline: 1.0925x; 1.0716x over previous
"""PVNet-style RANSAC keypoint voting (EvalWrapper) on 8 Trainium2 cores.

Split of work:
  host (jax CPU, bitwise-identical to the reference):
      mask/argmax, categorical sampling, hypothesis generation, winner
      refinement, and exact inlier-count recompute for a small candidate set.
  device (Bass, 8 NeuronCores):
      the O(HYP*VN*N) voting loop. Core c handles image c//4, hypotheses
      (c%4)*128..+128. A pixel votes for hypothesis h iff
      cos(angle(h - pix, dir)) > 0.99, which is equivalent to
          num > 0 and q > 0, where
          num = (h - pix) . dn                      (linear in (hx, hy, 1))
          q   = num^2 - T^2 * (|h - pix|^2 + eps)   (quadratic in hyp coords)
      Both are contractions of per-pixel coefficient vectors against per-hyp
      monomial vectors -> TensorE matmuls (contraction dims 6 and 3). Each
      f32 factor is split into two bf16s (hi + lo) and the contraction dim is
      expanded 3x (hi*hi + hi*lo + lo*hi), giving ~1e-5 relative accuracy at
      full bf16 PE speed. VectorE takes min(q, num); ScalarE Sign+accumulate
      produces per-hyp sums of sign(min) = 2*count - nfg_pad.
      Only foreground pixels are shipped (background never votes).

  The device counts are exact up to a few ULPs worth of borderline-cos pixel
  flips (measured max |delta| = 3). The host takes every hypothesis within
  MARGIN of the device max and recomputes its exact count with the reference's
  own jnp expressions (verified bitwise-identical to the full computation), so
  the final argmax winner -- including lowest-index tie-breaks -- matches the
  reference exactly, and the refined output is bit-for-bit the reference's.
"""

import numpy as np
import ml_dtypes

B, H, W, VN = 2, 64, 64, 9
HYP = 512
T = 0.99
T2 = T * T
EPS = 1e-6
N = H * W
NCORES = 8
CORES_PER_IMG = 4
HYP_PER_CORE = HYP // CORES_PER_IMG  # 128
PG = 1024                # pixels per partition-block group
MARGIN = 8.0

_BF16 = ml_dtypes.bfloat16

_nc_cache = {}
last_exec_time_ns = None   # set when BASS_KERNEL_TRACE=1
last_results = None


# ----------------------------------------------------------------------------
# host-side jax pieces (bitwise-identical to reference.py)
# ----------------------------------------------------------------------------

def _host_funcs():
    import jax
    import jax.numpy as jnp

    cpu = jax.devices("cpu")[0]

    def _perp(d):
        return jnp.stack([d[..., 1], -d[..., 0]], axis=-1)

    def run_part1(seg_pred, vertex_pred):
        with jax.default_device(cpu):
            seg_pred = jnp.asarray(np.asarray(seg_pred))
            vertex_pred = jnp.asarray(np.asarray(vertex_pred))
            vp = jnp.transpose(vertex_pred, (0, 2, 3, 1)).reshape(B, H, W, VN, 2)
            mask = jnp.argmax(seg_pred, axis=1)
            ys, xs = jnp.meshgrid(jnp.arange(H), jnp.arange(W), indexing="ij")
            coords = jnp.stack([xs, ys], -1).reshape(-1, 2).astype(jnp.float32)
            mflat = mask.reshape(B, -1).astype(jnp.float32)
            dflat = vp.reshape(B, H * W, VN, 2)
            keys = jax.random.split(jax.random.key(42), B)

            def part1(m, direct, key):
                logits = jnp.where(m > 0, 0.0, -1e9)
                idxs = jax.random.categorical(key, logits, shape=(HYP, VN, 2))
                ar = jnp.arange(VN)[None, :]
                p0 = coords[idxs[..., 0]]
                p1 = coords[idxs[..., 1]]
                d0 = direct[idxs[..., 0], ar]
                d1 = direct[idxs[..., 1], ar]
                n0, n1 = _perp(d0), _perp(d1)
                det = n0[..., 0] * n1[..., 1] - n0[..., 1] * n1[..., 0]
                det_s = jnp.where(jnp.abs(det) < EPS, EPS, det)
                c0 = jnp.sum(n0 * p0, -1)
                c1 = jnp.sum(n1 * p1, -1)
                hx = (c0 * n1[..., 1] - c1 * n0[..., 1]) / det_s
                hy = (n0[..., 0] * c1 - n1[..., 0] * c0) / det_s
                hyp = jnp.stack([hx, hy], -1)
                dn = direct / jnp.sqrt(jnp.sum(direct * direct, -1, keepdims=True) + EPS)
                return hyp, dn

            hyp_all, dn_all = jax.vmap(part1)(mflat, dflat, keys)
            return (np.asarray(hyp_all), np.asarray(dn_all), np.asarray(mflat),
                    np.asarray(dflat), np.asarray(coords))

    def exact_counts(hyp_c, dn_v, m, coords):
        """Reference-bitwise inlier counts for a subset of hyps of one (b, vn).

        hyp_c: [K, 2] f32, dn_v: [N, 2] f32, m: [N] f32 -> [K] f32
        """
        with jax.default_device(cpu):
            hyp_c = jnp.asarray(hyp_c)
            dn_v = jnp.asarray(dn_v)
            m = jnp.asarray(m)
            coords = jnp.asarray(coords)
            diff = hyp_c[:, None, :] - coords[None, :, :]
            dist = jnp.sqrt(jnp.sum(diff * diff, -1) + EPS)
            cos = jnp.einsum("knc,nc->kn", diff, dn_v) / dist
            inlier = (cos > T).astype(jnp.float32)
            return np.asarray(jnp.einsum("kn,n->k", inlier, m))

    def run_part3(hyp_all, dflat, dn_all, mflat, coords, win_idx):
        with jax.default_device(cpu):
            hyp_all = jnp.asarray(hyp_all)
            dflat = jnp.asarray(dflat)
            dn_all = jnp.asarray(dn_all)
            mflat = jnp.asarray(mflat)
            coords = jnp.asarray(coords)
            win_idx = jnp.asarray(win_idx)

            def part3(hyp, direct, dn, m, wi):
                win = hyp[wi, jnp.arange(VN)]
                dw = win[None, :, :] - coords[:, None, :]
                dwn = jnp.sqrt(jnp.sum(dw * dw, -1) + EPS)
                cw = jnp.sum(dw * dn, -1) / dwn
                wgt = ((cw > T) & (m[:, None] > 0)).astype(jnp.float32)
                normal = _perp(direct) * wgt[..., None]
                bvec = jnp.sum(normal * coords[:, None, :], -1)
                ATA = jnp.einsum("nvc,nvd->vcd", normal, normal)
                ATb = jnp.einsum("nvc,nv->vc", normal, bvec)
                detA = ATA[:, 0, 0] * ATA[:, 1, 1] - ATA[:, 0, 1] * ATA[:, 1, 0]
                detA_s = jnp.where(jnp.abs(detA) < EPS, EPS, detA)
                px = (ATA[:, 1, 1] * ATb[:, 0] - ATA[:, 0, 1] * ATb[:, 1]) / detA_s
                py = (-ATA[:, 1, 0] * ATb[:, 0] + ATA[:, 0, 0] * ATb[:, 1]) / detA_s
                refined = jnp.stack([px, py], -1)
                ok = (jnp.abs(detA) >= EPS)[:, None]
                return jnp.where(ok, refined, win)

            return np.asarray(jax.vmap(part3)(hyp_all, dflat, dn_all, mflat, win_idx))

    return run_part1, exact_counts, run_part3


# ----------------------------------------------------------------------------
# bf16x2 split packing
# ----------------------------------------------------------------------------

def _split_bf16(x):
    """f32/f64 array -> (hi, lo) bf16 arrays with hi + lo ~= f32(x)."""
    x32 = np.asarray(x, np.float32)
    hi = x32.astype(_BF16)
    lo = (x32 - hi.astype(np.float32)).astype(_BF16)
    return hi, lo


def _expand_rows(coeff, mono):
    """coeff [R, X], mono [R, Y] -> (rhs [3R, X], lhs [3R, Y]) bf16 such that
    sum_r lhs[r].T * rhs[r] ~= sum coeff*mono with ~2^-17 relative error:
    hi*hi + lo*hi + hi*lo (the lo*lo term is dropped)."""
    R, X = coeff.shape
    Y = mono.shape[1]
    ch, cl = _split_bf16(coeff)
    mh, ml = _split_bf16(mono)
    rhs = np.empty((3 * R, X), _BF16)
    lhs = np.empty((3 * R, Y), _BF16)
    rhs[0::3], rhs[1::3], rhs[2::3] = ch, cl, ch
    lhs[0::3], lhs[1::3], lhs[2::3] = mh, mh, ml
    return rhs, lhs


# ----------------------------------------------------------------------------
# device program
# ----------------------------------------------------------------------------

QROWS = 18   # 6 logical q-coefficients x3 bf16x2 expansion
NROWS = 3    # 3 logical num-coefficients, single bf16 (only its sign is used,
             # and it only matters when |num| > 0.99*dist >> bf16 error)
CROWS = QROWS + NROWS

# Partition layout (AP base partitions must be in {0, 32, 64}, and the DVE
# cannot read two PSUM operands in one op):
#   rows  0..17  q-coefficient rows, pixel group 0      (lhsT: q-monomials)
#   rows 32..49  q-coefficient rows, pixel group 1      (lhsT: q-monomials)
#   rows 64..72  num-coefficient rows, both groups      (lhsT: num-monomials)
# q matmuls contract 18 rows, num matmuls 9 rows; num groups are told apart
# by column ranges. Counting: ScalarE emits rn = relu(num) (PSUM->SBUF bf16),
# then one VectorE scalar_tensor_tensor computes
#   out = (q is_gt 0) logical_and rn  in {0.0, 1.0},  accum_out = count.
# rhs/lhs DMAs are batched 3 vns per transfer (the ~2us fixed cost per
# dma_start dominates small transfers).

MMC = 512                     # columns per matmul instruction (one PSUM bank)

def _build_bass(nfg_pad):
    import concourse.mybir as mybir
    from concourse import bacc, tile

    G = nfg_pad // PG            # pixel groups (pairs: g even at rows 0, odd at 32)
    assert nfg_pad % (2 * PG) == 0, nfg_pad
    NMM = PG // MMC              # matmuls per (group, q|num)

    nc = bacc.Bacc("TRN2", target_bir_lowering=False, debug=False)
    lhs_d = nc.declare_dram_parameter("lhs", [128, VN * 128], mybir.dt.bfloat16,
                                      isOutput=False)
    rq0_d = nc.declare_dram_parameter("rhsq0", [QROWS, VN * (G // 2) * PG],
                                      mybir.dt.bfloat16, isOutput=False)
    rq1_d = nc.declare_dram_parameter("rhsq1", [QROWS, VN * (G // 2) * PG],
                                      mybir.dt.bfloat16, isOutput=False)
    rn_d = nc.declare_dram_parameter("rhsn", [NROWS, VN * G * PG],
                                     mybir.dt.bfloat16, isOutput=False)
    acc_d = nc.declare_dram_parameter("acc", [128, VN * G], mybir.dt.float32,
                                      isOutput=True)

    QCOLS = (G // 2) * PG        # q columns per vn per parity block
    NCOLS = G * PG               # num columns per vn

    with tile.TileContext(nc) as tc:
        with (
            tc.tile_pool(name="const", bufs=1) as cpool,
            tc.tile_pool(name="rhs", bufs=4) as rpool,
            tc.tile_pool(name="sn", bufs=4) as snpool,
            tc.tile_pool(name="pr", bufs=4) as prpool,
            tc.tile_pool(name="psq", bufs=2, space="PSUM") as psq,
            tc.tile_pool(name="psn", bufs=2, space="PSUM") as psn,
        ):
            lhs_t = cpool.tile([128, VN * 128], mybir.dt.bfloat16)
            nc.sync.dma_start(lhs_t[0:73, :], lhs_d[0:73, :])
            acc_t = cpool.tile([128, VN * G], mybir.dt.float32)

            for vn in range(VN):
                rhs_t = rpool.tile([128, QCOLS + NCOLS], mybir.dt.bfloat16)
                qs = slice(vn * QCOLS, (vn + 1) * QCOLS)
                ns = slice(vn * NCOLS, (vn + 1) * NCOLS)
                # three parallel DMA lanes: Sync HWDGE / Act HWDGE / SWDGE
                nc.sync.dma_start(rhs_t[0:QROWS, 0:QCOLS], rq0_d[:, qs])
                nc.scalar.dma_start(rhs_t[32:32 + QROWS, 0:QCOLS], rq1_d[:, qs])
                nc.gpsimd.dma_start(
                    rhs_t[64:64 + NROWS, QCOLS:QCOLS + NCOLS], rn_d[:, ns])
                for g in range(G):
                    qb = 32 * (g % 2)
                    qco = (g // 2) * PG
                    nco = QCOLS + g * PG
                    qp = psq.tile([128, PG], mybir.dt.float32)
                    np_ = psn.tile([128, PG], mybir.dt.float32)
                    for s in range(NMM):
                        nc.tensor.matmul(
                            qp[:, s * MMC:(s + 1) * MMC],
                            lhs_t[qb:qb + QROWS, vn * 128:(vn + 1) * 128],
                            rhs_t[qb:qb + QROWS,
                                  qco + s * MMC:qco + (s + 1) * MMC],
                            start=True, stop=True)
                        nc.tensor.matmul(
                            np_[:, s * MMC:(s + 1) * MMC],
                            lhs_t[64:64 + NROWS, vn * 128:(vn + 1) * 128],
                            rhs_t[64:64 + NROWS,
                                  nco + s * MMC:nco + (s + 1) * MMC],
                            start=True, stop=True)
                    slot = vn * G + g
                    sn_t = snpool.tile([128, PG], mybir.dt.bfloat16)
                    nc.scalar.activation(sn_t[:], np_[:],
                                         mybir.ActivationFunctionType.Relu)
                    st_t = prpool.tile([128, PG], mybir.dt.bfloat16)
                    nc.vector.scalar_tensor_tensor(
                        st_t[:], qp[:], 0.0, sn_t[:],
                        mybir.AluOpType.is_gt, mybir.AluOpType.logical_and,
                        accum_out=acc_t[:, slot:slot + 1])
            nc.sync.dma_start(acc_d[:], acc_t[:])
    return nc


def _get_nc(nfg_pad):
    if nfg_pad not in _nc_cache:
        nc = _build_bass(nfg_pad)
        if not nc.is_finalized():
            nc.finalize()
        _nc_cache[nfg_pad] = nc
    return _nc_cache[nfg_pad]


# ----------------------------------------------------------------------------
# per-core input packing
# ----------------------------------------------------------------------------

def _pack_image_rhs(fg_idx, coords, dn_img, nfg_pad):
    """rhs for one image -> dict with rhsq0 [18, VN*QCOLS], rhsq1 [18, VN*QCOLS],
    rhsn [9, VN*NCOLS] bf16."""
    G = nfg_pad // PG
    QCOLS = (G // 2) * PG
    NCOLS = G * PG
    nfg = len(fg_idx)
    cx = coords[fg_idx, 0].astype(np.float64)
    cy = coords[fg_idx, 1].astype(np.float64)

    rq0 = np.zeros((QROWS, VN * QCOLS), _BF16)
    rq1 = np.zeros((QROWS, VN * QCOLS), _BF16)
    rn = np.zeros((NROWS, VN * NCOLS), _BF16)
    for vn in range(VN):
        A = dn_img[fg_idx, vn, 0].astype(np.float64)
        Bc = dn_img[fg_idx, vn, 1].astype(np.float64)
        C = -(cx * A + cy * Bc)
        q6 = np.zeros((6, nfg_pad), np.float64)
        n3 = np.zeros((3, nfg_pad), np.float64)
        q6[0, :nfg] = A * A - T2
        q6[1, :nfg] = 2.0 * A * Bc
        q6[2, :nfg] = Bc * Bc - T2
        q6[3, :nfg] = 2.0 * A * C + 2.0 * T2 * cx
        q6[4, :nfg] = 2.0 * Bc * C + 2.0 * T2 * cy
        q6[5, :nfg] = C * C - T2 * (cx * cx + cy * cy + EPS)
        n3[0, :nfg] = A
        n3[1, :nfg] = Bc
        n3[2, :nfg] = C
        # padding pixels: q = -1, num = -1  (constant rows; monomial row = 1)
        q6[5, nfg:] = -1.0
        n3[2, nfg:] = -1.0

        rhs_q, _ = _expand_rows(q6, np.zeros((6, 1)))   # [18, nfg_pad]
        rhs_n = n3.astype(np.float32).astype(_BF16)     # [3, nfg_pad]
        for g in range(G):
            sl = slice(g * PG, (g + 1) * PG)
            dst = rq0 if g % 2 == 0 else rq1
            qc = vn * QCOLS + (g // 2) * PG
            dst[:, qc:qc + PG] = rhs_q[:, sl]
            nc_ = vn * NCOLS + g * PG
            rn[:, nc_:nc_ + PG] = rhs_n[:, sl]
    return {"rhsq0": rq0, "rhsq1": rq1, "rhsn": rn}


def _pack_core_lhs(hyp_blk):
    """lhs for one core: [128, VN * 128] bf16. hyp_blk: [128, VN, 2] f32."""
    lhs_all = np.zeros((128, VN * 128), _BF16)
    for vn in range(VN):
        hx = hyp_blk[:, vn, 0].astype(np.float64)
        hy = hyp_blk[:, vn, 1].astype(np.float64)
        one = np.ones_like(hx)
        m6 = np.stack([hx * hx, hx * hy, hy * hy, hx, hy, one])   # [6, 128]
        m3 = np.stack([hx, hy, one])                              # [3, 128]
        _, lhs_q = _expand_rows(np.zeros((6, 1)), m6)
        lhs_n = m3.astype(np.float32).astype(_BF16)
        lhs_all[0:QROWS, vn * 128:(vn + 1) * 128] = lhs_q
        lhs_all[32:32 + QROWS, vn * 128:(vn + 1) * 128] = lhs_q
        lhs_all[64:64 + NROWS, vn * 128:(vn + 1) * 128] = lhs_n
    return lhs_all


# ----------------------------------------------------------------------------
# main entry
# ----------------------------------------------------------------------------

def kernel(seg_pred, vertex_pred):
    global last_exec_time_ns, last_results
    import os

    run_part1, exact_counts, run_part3 = _host_funcs()
    hyp_all, dn_all, mflat, dflat, coords = run_part1(seg_pred, vertex_pred)

    fg = [np.nonzero(mflat[b] > 0)[0] for b in range(B)]
    maxfg = max(len(f) for f in fg)
    nfg_pad = max(2048, ((maxfg + 2047) // 2048) * 2048)
    G = nfg_pad // PG
    NACC = G

    nc = _get_nc(nfg_pad)

    rhs_img = [_pack_image_rhs(fg[b], coords, dn_all[b], nfg_pad) for b in range(B)]
    in_maps = []
    for c in range(NCORES):
        b = c // CORES_PER_IMG
        k = c % CORES_PER_IMG
        hyp_blk = hyp_all[b, k * HYP_PER_CORE:(k + 1) * HYP_PER_CORE]
        in_maps.append({"lhs": np.ascontiguousarray(_pack_core_lhs(hyp_blk)),
                        **rhs_img[b]})

    from concourse.bass_utils import run_bass_kernel_spmd
    trace = os.environ.get("BASS_KERNEL_TRACE", "0") == "1"
    res = run_bass_kernel_spmd(nc, in_maps, core_ids=list(range(NCORES)),
                               trace=trace)
    last_exec_time_ns = res.exec_time_ns
    last_results = res

    # assemble device counts [B, HYP, VN]: acc slots are direct counts per group
    dev_counts = np.zeros((B, HYP, VN), np.float64)
    for c in range(NCORES):
        b = c // CORES_PER_IMG
        k = c % CORES_PER_IMG
        acc = np.asarray(res.results[c]["acc"], np.float64)  # [128, VN*G]
        dev_counts[b, k * HYP_PER_CORE:(k + 1) * HYP_PER_CORE] = \
            acc.reshape(128, VN, G).sum(-1)

    # winner selection: exact recompute of candidates (reference-bitwise)
    win_idx = np.zeros((B, VN), np.int64)
    for b in range(B):
        for vn in range(VN):
            dc = dev_counts[b, :, vn]
            cand = np.nonzero(dc >= dc.max() - MARGIN)[0]
            cnt = exact_counts(hyp_all[b][cand, vn, :], dn_all[b][:, vn, :],
                               mflat[b], coords)
            win_idx[b, vn] = cand[int(np.argmax(cnt))]

    out = run_part3(hyp_all, dflat, dn_all, mflat, coords, win_idx)
    return np.asarray(out, np.float32)


# revision 39
# speedup vs baseline: 1.1030x; 1.0096x over previous
"""PVNet-style RANSAC keypoint voting (EvalWrapper) on 8 Trainium2 cores.

Split of work:
  host (jax CPU, bitwise-identical to the reference):
      mask/argmax, categorical sampling, hypothesis generation, winner
      refinement, and exact inlier-count recompute for a small candidate set.
  device (Bass, 8 NeuronCores):
      the O(HYP*VN*N) voting loop. Core c handles image c//4, hypotheses
      (c%4)*128..+128. A pixel votes for hypothesis h iff
      cos(angle(h - pix, dir)) > 0.99, which is equivalent to
          num > 0 and q > 0, where
          num = (h - pix) . dn                      (linear in (hx, hy, 1))
          q   = num^2 - T^2 * (|h - pix|^2 + eps)   (quadratic in hyp coords)
      Both are contractions of per-pixel coefficient vectors against per-hyp
      monomial vectors -> TensorE matmuls (contraction dims 6 and 3). Each
      f32 factor is split into two bf16s (hi + lo) and the contraction dim is
      expanded 3x (hi*hi + hi*lo + lo*hi), giving ~1e-5 relative accuracy at
      full bf16 PE speed. VectorE takes min(q, num); ScalarE Sign+accumulate
      produces per-hyp sums of sign(min) = 2*count - nfg_pad.
      Only foreground pixels are shipped (background never votes).

  The device counts are exact up to a few ULPs worth of borderline-cos pixel
  flips (measured max |delta| = 3). The host takes every hypothesis within
  MARGIN of the device max and recomputes its exact count with the reference's
  own jnp expressions (verified bitwise-identical to the full computation), so
  the final argmax winner -- including lowest-index tie-breaks -- matches the
  reference exactly, and the refined output is bit-for-bit the reference's.
"""

import numpy as np
import ml_dtypes

B, H, W, VN = 2, 64, 64, 9
HYP = 512
T = 0.99
T2 = T * T
EPS = 1e-6
N = H * W
NCORES = 8
NBLK = 4                 # hypothesis blocks of 128 per image
PG = 1024                # pixels per partition-block group
MARGIN = 8.0

# Work is split into 72 units (b, vn, blk): blk = 128 hypotheses. Each core
# runs 9 units: all 4 blocks of two full (b, vn) pixel-slices plus one block
# of a shared slice — a uniform [4, 4, 1] pattern so the SPMD program is
# identical across cores while each core only loads ~2.25 pixel-slices.
_SLICES = [(b, vn) for b in range(B) for vn in range(VN)]  # 18

def _core_units(c):
    s0, s1 = _SLICES[2 * c], _SLICES[2 * c + 1]
    ssplit = _SLICES[16 + c // 4]
    blk = c % 4
    return ([(s0[0], s0[1], k) for k in range(NBLK)]
            + [(s1[0], s1[1], k) for k in range(NBLK)]
            + [(ssplit[0], ssplit[1], blk)])

def _core_slices(c):
    return [_SLICES[2 * c], _SLICES[2 * c + 1], _SLICES[16 + c // 4]]

_BF16 = ml_dtypes.bfloat16

_nc_cache = {}
last_exec_time_ns = None   # set when BASS_KERNEL_TRACE=1
last_results = None


# ----------------------------------------------------------------------------
# host-side jax pieces (bitwise-identical to reference.py)
# ----------------------------------------------------------------------------

def _host_funcs():
    import jax
    import jax.numpy as jnp

    cpu = jax.devices("cpu")[0]

    def _perp(d):
        return jnp.stack([d[..., 1], -d[..., 0]], axis=-1)

    def run_part1(seg_pred, vertex_pred):
        with jax.default_device(cpu):
            seg_pred = jnp.asarray(np.asarray(seg_pred))
            vertex_pred = jnp.asarray(np.asarray(vertex_pred))
            vp = jnp.transpose(vertex_pred, (0, 2, 3, 1)).reshape(B, H, W, VN, 2)
            mask = jnp.argmax(seg_pred, axis=1)
            ys, xs = jnp.meshgrid(jnp.arange(H), jnp.arange(W), indexing="ij")
            coords = jnp.stack([xs, ys], -1).reshape(-1, 2).astype(jnp.float32)
            mflat = mask.reshape(B, -1).astype(jnp.float32)
            dflat = vp.reshape(B, H * W, VN, 2)
            keys = jax.random.split(jax.random.key(42), B)

            def part1(m, direct, key):
                logits = jnp.where(m > 0, 0.0, -1e9)
                idxs = jax.random.categorical(key, logits, shape=(HYP, VN, 2))
                ar = jnp.arange(VN)[None, :]
                p0 = coords[idxs[..., 0]]
                p1 = coords[idxs[..., 1]]
                d0 = direct[idxs[..., 0], ar]
                d1 = direct[idxs[..., 1], ar]
                n0, n1 = _perp(d0), _perp(d1)
                det = n0[..., 0] * n1[..., 1] - n0[..., 1] * n1[..., 0]
                det_s = jnp.where(jnp.abs(det) < EPS, EPS, det)
                c0 = jnp.sum(n0 * p0, -1)
                c1 = jnp.sum(n1 * p1, -1)
                hx = (c0 * n1[..., 1] - c1 * n0[..., 1]) / det_s
                hy = (n0[..., 0] * c1 - n1[..., 0] * c0) / det_s
                hyp = jnp.stack([hx, hy], -1)
                dn = direct / jnp.sqrt(jnp.sum(direct * direct, -1, keepdims=True) + EPS)
                return hyp, dn

            hyp_all, dn_all = jax.vmap(part1)(mflat, dflat, keys)
            return (np.asarray(hyp_all), np.asarray(dn_all), np.asarray(mflat),
                    np.asarray(dflat), np.asarray(coords))

    def exact_counts(hyp_c, dn_v, m, coords):
        """Reference-bitwise inlier counts for a subset of hyps of one (b, vn).

        hyp_c: [K, 2] f32, dn_v: [N, 2] f32, m: [N] f32 -> [K] f32
        """
        with jax.default_device(cpu):
            hyp_c = jnp.asarray(hyp_c)
            dn_v = jnp.asarray(dn_v)
            m = jnp.asarray(m)
            coords = jnp.asarray(coords)
            diff = hyp_c[:, None, :] - coords[None, :, :]
            dist = jnp.sqrt(jnp.sum(diff * diff, -1) + EPS)
            cos = jnp.einsum("knc,nc->kn", diff, dn_v) / dist
            inlier = (cos > T).astype(jnp.float32)
            return np.asarray(jnp.einsum("kn,n->k", inlier, m))

    def run_part3(hyp_all, dflat, dn_all, mflat, coords, win_idx):
        with jax.default_device(cpu):
            hyp_all = jnp.asarray(hyp_all)
            dflat = jnp.asarray(dflat)
            dn_all = jnp.asarray(dn_all)
            mflat = jnp.asarray(mflat)
            coords = jnp.asarray(coords)
            win_idx = jnp.asarray(win_idx)

            def part3(hyp, direct, dn, m, wi):
                win = hyp[wi, jnp.arange(VN)]
                dw = win[None, :, :] - coords[:, None, :]
                dwn = jnp.sqrt(jnp.sum(dw * dw, -1) + EPS)
                cw = jnp.sum(dw * dn, -1) / dwn
                wgt = ((cw > T) & (m[:, None] > 0)).astype(jnp.float32)
                normal = _perp(direct) * wgt[..., None]
                bvec = jnp.sum(normal * coords[:, None, :], -1)
                ATA = jnp.einsum("nvc,nvd->vcd", normal, normal)
                ATb = jnp.einsum("nvc,nv->vc", normal, bvec)
                detA = ATA[:, 0, 0] * ATA[:, 1, 1] - ATA[:, 0, 1] * ATA[:, 1, 0]
                detA_s = jnp.where(jnp.abs(detA) < EPS, EPS, detA)
                px = (ATA[:, 1, 1] * ATb[:, 0] - ATA[:, 0, 1] * ATb[:, 1]) / detA_s
                py = (-ATA[:, 1, 0] * ATb[:, 0] + ATA[:, 0, 0] * ATb[:, 1]) / detA_s
                refined = jnp.stack([px, py], -1)
                ok = (jnp.abs(detA) >= EPS)[:, None]
                return jnp.where(ok, refined, win)

            return np.asarray(jax.vmap(part3)(hyp_all, dflat, dn_all, mflat, win_idx))

    return run_part1, exact_counts, run_part3


# ----------------------------------------------------------------------------
# bf16x2 split packing
# ----------------------------------------------------------------------------

def _split_bf16(x):
    """f32/f64 array -> (hi, lo) bf16 arrays with hi + lo ~= f32(x)."""
    x32 = np.asarray(x, np.float32)
    hi = x32.astype(_BF16)
    lo = (x32 - hi.astype(np.float32)).astype(_BF16)
    return hi, lo


def _expand_rows(coeff, mono):
    """coeff [R, X], mono [R, Y] -> (rhs [3R, X], lhs [3R, Y]) bf16 such that
    sum_r lhs[r].T * rhs[r] ~= sum coeff*mono with ~2^-17 relative error:
    hi*hi + lo*hi + hi*lo (the lo*lo term is dropped)."""
    R, X = coeff.shape
    Y = mono.shape[1]
    ch, cl = _split_bf16(coeff)
    mh, ml = _split_bf16(mono)
    rhs = np.empty((3 * R, X), _BF16)
    lhs = np.empty((3 * R, Y), _BF16)
    rhs[0::3], rhs[1::3], rhs[2::3] = ch, cl, ch
    lhs[0::3], lhs[1::3], lhs[2::3] = mh, mh, ml
    return rhs, lhs


# ----------------------------------------------------------------------------
# device program
# ----------------------------------------------------------------------------

QROWS = 18   # 6 logical q-coefficients x3 bf16x2 expansion
NROWS = 3    # 3 logical num-coefficients, single bf16 (only its sign is used,
             # and it only matters when |num| > 0.99*dist >> bf16 error)
CROWS = QROWS + NROWS

# Partition layout (AP base partitions must be in {0, 32, 64}, and the DVE
# cannot read two PSUM operands in one op):
#   rows  0..17  q-coefficient rows, pixel group 0      (lhsT: q-monomials)
#   rows 32..49  q-coefficient rows, pixel group 1      (lhsT: q-monomials)
#   rows 64..72  num-coefficient rows, both groups      (lhsT: num-monomials)
# q matmuls contract 18 rows, num matmuls 9 rows; num groups are told apart
# by column ranges. Counting: ScalarE emits rn = relu(num) (PSUM->SBUF bf16),
# then one VectorE scalar_tensor_tensor computes
#   out = (q is_gt 0) logical_and rn  in {0.0, 1.0},  accum_out = count.
# rhs/lhs DMAs are batched 3 vns per transfer (the ~2us fixed cost per
# dma_start dominates small transfers).

MMC = 512                     # columns per matmul instruction (one PSUM bank)

def _build_bass(nfg_pad):
    import concourse.mybir as mybir
    from concourse import bacc, tile

    G = nfg_pad // PG            # pixel groups (pairs: g even at rows 0, odd at 32)
    assert nfg_pad % (2 * PG) == 0, nfg_pad
    NMM = PG // MMC              # matmuls per (group, q|num)

    NUNITS = 2 * NBLK + 1        # 9 units per core
    NSL = 3                      # pixel-slices per core
    nc = bacc.Bacc("TRN2", target_bir_lowering=False, debug=False)
    lhs_d = nc.declare_dram_parameter("lhs", [128, NUNITS * 128],
                                      mybir.dt.bfloat16, isOutput=False)
    rq0_d = nc.declare_dram_parameter("rhsq0", [QROWS, NSL * (G // 2) * PG],
                                      mybir.dt.bfloat16, isOutput=False)
    rq1_d = nc.declare_dram_parameter("rhsq1", [QROWS, NSL * (G // 2) * PG],
                                      mybir.dt.bfloat16, isOutput=False)
    rn_d = nc.declare_dram_parameter("rhsn", [NROWS, NSL * G * PG],
                                     mybir.dt.bfloat16, isOutput=False)
    acc_d = nc.declare_dram_parameter("acc", [128, NUNITS * G], mybir.dt.float32,
                                      isOutput=True)

    QCOLS = (G // 2) * PG        # q columns per slice per parity block
    NCOLS = G * PG               # num columns per slice
    ACC_SPLIT = 5 * G            # dma the first 5 units' counts early

    with tile.TileContext(nc) as tc:
        with (
            tc.tile_pool(name="const", bufs=1) as cpool,
            tc.tile_pool(name="rhs", bufs=2) as rpool,
            tc.tile_pool(name="sn", bufs=4) as snpool,
            tc.tile_pool(name="pr", bufs=4) as prpool,
            tc.tile_pool(name="psq", bufs=2, space="PSUM") as psq,
            tc.tile_pool(name="psn", bufs=2, space="PSUM") as psn,
        ):
            # lhs in two tiles so unit 0 doesn't wait for the full transfer
            lhsA = cpool.tile([128, NBLK * 128], mybir.dt.bfloat16)
            nc.sync.dma_start(lhsA[0:73, :], lhs_d[0:73, 0:NBLK * 128])
            lhsB = cpool.tile([128, (NBLK + 1) * 128], mybir.dt.bfloat16)
            nc.sync.dma_start(lhsB[0:73, :], lhs_d[0:73, NBLK * 128:])
            acc_t = cpool.tile([128, NUNITS * G], mybir.dt.float32)

            for j in range(NUNITS):
                sl = j // NBLK if j < 2 * NBLK else 2
                if j % NBLK == 0:
                    rhs_t = rpool.tile([128, QCOLS + NCOLS], mybir.dt.bfloat16)
                    qs = slice(sl * QCOLS, (sl + 1) * QCOLS)
                    ns = slice(sl * NCOLS, (sl + 1) * NCOLS)
                    # three parallel DMA lanes: Sync HWDGE / Act HWDGE / SWDGE
                    nc.sync.dma_start(rhs_t[0:QROWS, 0:QCOLS], rq0_d[:, qs])
                    nc.scalar.dma_start(rhs_t[32:32 + QROWS, 0:QCOLS],
                                        rq1_d[:, qs])
                    nc.gpsimd.dma_start(
                        rhs_t[64:64 + NROWS, QCOLS:QCOLS + NCOLS], rn_d[:, ns])
                lhs_t = lhsA if j < NBLK else lhsB
                lc = j * 128 if j < NBLK else (j - NBLK) * 128
                for g in range(G):
                    qb = 32 * (g % 2)
                    qco = (g // 2) * PG
                    nco = QCOLS + g * PG
                    qp = psq.tile([128, PG], mybir.dt.float32)
                    np_ = psn.tile([128, PG], mybir.dt.float32)
                    for s in range(NMM):
                        nc.tensor.matmul(
                            qp[:, s * MMC:(s + 1) * MMC],
                            lhs_t[qb:qb + QROWS, lc:lc + 128],
                            rhs_t[qb:qb + QROWS,
                                  qco + s * MMC:qco + (s + 1) * MMC],
                            start=True, stop=True)
                        nc.tensor.matmul(
                            np_[:, s * MMC:(s + 1) * MMC],
                            lhs_t[64:64 + NROWS, lc:lc + 128],
                            rhs_t[64:64 + NROWS,
                                  nco + s * MMC:nco + (s + 1) * MMC],
                            start=True, stop=True)
                    slot = j * G + g
                    sn_t = snpool.tile([128, PG], mybir.dt.bfloat16)
                    nc.scalar.activation(sn_t[:], np_[:],
                                         mybir.ActivationFunctionType.Relu)
                    st_t = prpool.tile([128, PG], mybir.dt.bfloat16)
                    nc.vector.scalar_tensor_tensor(
                        st_t[:], qp[:], 0.0, sn_t[:],
                        mybir.AluOpType.is_gt, mybir.AluOpType.logical_and,
                        accum_out=acc_t[:, slot:slot + 1])
            nc.sync.dma_start(acc_d[:, 0:ACC_SPLIT], acc_t[:, 0:ACC_SPLIT])
            nc.sync.dma_start(acc_d[:, ACC_SPLIT:], acc_t[:, ACC_SPLIT:])
    return nc


def _get_nc(nfg_pad):
    if nfg_pad not in _nc_cache:
        nc = _build_bass(nfg_pad)
        if not nc.is_finalized():
            nc.finalize()
        _nc_cache[nfg_pad] = nc
    return _nc_cache[nfg_pad]


# ----------------------------------------------------------------------------
# per-core input packing
# ----------------------------------------------------------------------------

def _pack_slice_rhs(fg_idx, coords, dn_img, vn, nfg_pad):
    """rhs for one (image, vn) slice -> (rq0 [18, QCOLS], rq1 [18, QCOLS],
    rn [3, NCOLS]) bf16."""
    G = nfg_pad // PG
    QCOLS = (G // 2) * PG
    NCOLS = G * PG
    nfg = len(fg_idx)
    cx = coords[fg_idx, 0].astype(np.float64)
    cy = coords[fg_idx, 1].astype(np.float64)

    rq0 = np.zeros((QROWS, QCOLS), _BF16)
    rq1 = np.zeros((QROWS, QCOLS), _BF16)
    rn = np.zeros((NROWS, NCOLS), _BF16)
    A = dn_img[fg_idx, vn, 0].astype(np.float64)
    Bc = dn_img[fg_idx, vn, 1].astype(np.float64)
    C = -(cx * A + cy * Bc)
    q6 = np.zeros((6, nfg_pad), np.float64)
    n3 = np.zeros((3, nfg_pad), np.float64)
    q6[0, :nfg] = A * A - T2
    q6[1, :nfg] = 2.0 * A * Bc
    q6[2, :nfg] = Bc * Bc - T2
    q6[3, :nfg] = 2.0 * A * C + 2.0 * T2 * cx
    q6[4, :nfg] = 2.0 * Bc * C + 2.0 * T2 * cy
    q6[5, :nfg] = C * C - T2 * (cx * cx + cy * cy + EPS)
    n3[0, :nfg] = A
    n3[1, :nfg] = Bc
    n3[2, :nfg] = C
    # padding pixels: q = -1, num = -1  (constant rows; monomial row = 1)
    q6[5, nfg:] = -1.0
    n3[2, nfg:] = -1.0

    rhs_q, _ = _expand_rows(q6, np.zeros((6, 1)))   # [18, nfg_pad]
    rhs_n = n3.astype(np.float32).astype(_BF16)     # [3, nfg_pad]
    for g in range(G):
        sl = slice(g * PG, (g + 1) * PG)
        dst = rq0 if g % 2 == 0 else rq1
        qc = (g // 2) * PG
        dst[:, qc:qc + PG] = rhs_q[:, sl]
        rn[:, g * PG:(g + 1) * PG] = rhs_n[:, sl]
    return rq0, rq1, rn


def _pack_unit_lhs(hyps):
    """lhs columns for one unit: [128, 128] bf16. hyps: [128, 2] f32."""
    out = np.zeros((128, 128), _BF16)
    hx = hyps[:, 0].astype(np.float64)
    hy = hyps[:, 1].astype(np.float64)
    one = np.ones_like(hx)
    m6 = np.stack([hx * hx, hx * hy, hy * hy, hx, hy, one])   # [6, 128]
    m3 = np.stack([hx, hy, one])                              # [3, 128]
    _, lhs_q = _expand_rows(np.zeros((6, 1)), m6)
    lhs_n = m3.astype(np.float32).astype(_BF16)
    out[0:QROWS, :] = lhs_q
    out[32:32 + QROWS, :] = lhs_q
    out[64:64 + NROWS, :] = lhs_n
    return out


# ----------------------------------------------------------------------------
# main entry
# ----------------------------------------------------------------------------

def kernel(seg_pred, vertex_pred):
    global last_exec_time_ns, last_results
    import os

    run_part1, exact_counts, run_part3 = _host_funcs()
    hyp_all, dn_all, mflat, dflat, coords = run_part1(seg_pred, vertex_pred)

    fg = [np.nonzero(mflat[b] > 0)[0] for b in range(B)]
    maxfg = max(len(f) for f in fg)
    nfg_pad = max(2048, ((maxfg + 2047) // 2048) * 2048)
    G = nfg_pad // PG
    NACC = G

    nc = _get_nc(nfg_pad)

    slice_rhs = {}
    for (b, vn) in _SLICES:
        slice_rhs[(b, vn)] = _pack_slice_rhs(fg[b], coords, dn_all[b], vn,
                                             nfg_pad)

    in_maps = []
    for c in range(NCORES):
        units = _core_units(c)
        lhs = np.zeros((128, len(units) * 128), _BF16)
        for j, (b, vn, blk) in enumerate(units):
            lhs[:, j * 128:(j + 1) * 128] = _pack_unit_lhs(
                hyp_all[b, blk * 128:(blk + 1) * 128, vn, :])
        sls = _core_slices(c)
        rq0 = np.concatenate([slice_rhs[s][0] for s in sls], axis=1)
        rq1 = np.concatenate([slice_rhs[s][1] for s in sls], axis=1)
        rn = np.concatenate([slice_rhs[s][2] for s in sls], axis=1)
        in_maps.append({"lhs": np.ascontiguousarray(lhs),
                        "rhsq0": np.ascontiguousarray(rq0),
                        "rhsq1": np.ascontiguousarray(rq1),
                        "rhsn": np.ascontiguousarray(rn)})

    from concourse.bass_utils import run_bass_kernel_spmd
    trace = os.environ.get("BASS_KERNEL_TRACE", "0") == "1"
    res = run_bass_kernel_spmd(nc, in_maps, core_ids=list(range(NCORES)),
                               trace=trace)
    last_exec_time_ns = res.exec_time_ns
    last_results = res

    # assemble device counts [B, HYP, VN]: acc slots are direct counts per group
    dev_counts = np.zeros((B, HYP, VN), np.float64)
    for c in range(NCORES):
        acc = np.asarray(res.results[c]["acc"], np.float64)  # [128, NUNITS*G]
        acc = acc.reshape(128, len(_core_units(c)), G).sum(-1)
        for j, (b, vn, blk) in enumerate(_core_units(c)):
            dev_counts[b, blk * 128:(blk + 1) * 128, vn] = acc[:, j]

    # winner selection: exact recompute of candidates (reference-bitwise)
    win_idx = np.zeros((B, VN), np.int64)
    for b in range(B):
        for vn in range(VN):
            dc = dev_counts[b, :, vn]
            cand = np.nonzero(dc >= dc.max() - MARGIN)[0]
            cnt = exact_counts(hyp_all[b][cand, vn, :], dn_all[b][:, vn, :],
                               mflat[b], coords)
            win_idx[b, vn] = cand[int(np.argmax(cnt))]

    out = run_part3(hyp_all, dflat, dn_all, mflat, coords, win_idx)
    return np.asarray(out, np.float32)
